# revision 49
# baseline (speedup 1.0000x reference)
"""Multi-head attention block (B=16, N=577, C=1024, H=16) on 8 Trainium2 NeuronCores.

Sharding: data-parallel over batch — 2 batch elements per core, no collectives.

Device dataflow per batch element (fully "transposed" so no on-device transposes):
  inputs staged host-side: xT = x^T  [C,N] bf16, wqkvT = w_qkv^T [C,3C] bf16,
  wprojT = w_proj^T [C,C] bf16.
  qT,kT [o,n] <- (wqkvT tile).T @ xT      (o on partitions: per-head [64, n])
  V     [n,o] <- (xT tile).T @ wqkvT      (n on partitions: per-head [m, 64])
  S^T   [m,n] <- (kT_h tile [d,m]).T @ qT_h [d,n]          (d=64 contraction)
  P^T = exp(0.125 * S^T)                  (softmax numerator; max-subtraction skipped:
                                           scaled scores are ~N(0,1), |s|<~10, exp safe)
  OT'[0:64,n] = sum_m V_h[m,d] P^T[m,n];  OT'[64,n] = sum_m P^T[m,n]
      (one matmul: lhsT = [V_h | ones] [m, 65] — sumexp comes free as row 64)
  OT = OT'[0:64] * (1/OT'[64])            (softmax denominator)
  y[n,o] = (OT tile [c,n]).T @ wprojT + b_proj
"""

import os
import sys

import numpy as np

if "/opt/trn_rl_repo" not in sys.path:
    sys.path.insert(0, "/opt/trn_rl_repo")

import ml_dtypes

B, N, C = 16, 577, 1024
H, D = 16, 64
P = 128
CT = C // P  # 8 contraction tiles
NT = 5  # n(row) tiles of 128: 4*128 + 65
NTS = [128, 128, 128, 128, 65]
NCH = [(0, 512), (512, 65)]  # free-dim chunks of 577 (psum bank = 512 fp32)
NCORES = 8
BPC = B // NCORES  # batches per core

_CACHE = {}
LAST_RESULT = None


def _ensure_ntff_hook():
    """Install antenv.axon_hooks with a ctypes-based NTFF profile hook if the
    environment's antenv package lacks it (mirrors trn_boot._ntff_profile_via_ctypes).
    Without this, run_bass_kernel_spmd(trace=True) silently skips tracing."""
    try:
        from antenv import axon_hooks  # noqa: F401

        return
    except ImportError:
        pass
    import contextlib
    import ctypes
    import types

    import antenv

    so_path = "/opt/axon/libaxon_pjrt.so"
    mod = types.ModuleType("antenv.axon_hooks")
    _state = {"hook": None, "set": False}

    def _make_hook():
        if not os.path.exists(so_path):
            return None
        lib = ctypes.CDLL(so_path)
        if not hasattr(lib, "axon_start_nrt_profile"):
            return None
        lib.axon_start_nrt_profile.argtypes = [
            ctypes.POINTER(ctypes.c_int64),
            ctypes.c_size_t,
        ]
        lib.axon_start_nrt_profile.restype = ctypes.c_int64
        lib.axon_stop_nrt_profile.argtypes = [ctypes.c_char_p]
        lib.axon_stop_nrt_profile.restype = ctypes.c_int64

        @contextlib.contextmanager
        def _hook(output_dir, device_ids):
            import jax

            jax.devices()
            if device_ids:
                ids = (ctypes.c_int64 * len(device_ids))(*device_ids)
                rc = lib.axon_start_nrt_profile(ids, len(device_ids))
            else:
                rc = lib.axon_start_nrt_profile(None, 0)
            if rc != 0:
                raise RuntimeError(f"axon_start_nrt_profile rc={rc}")
            try:
                yield
            finally:
                n = lib.axon_stop_nrt_profile(str(output_dir).encode())
                print(f"ntff profile: {n} file(s) written to {output_dir}", file=sys.stderr)

        return _hook

    def set_axon_ntff_profile_hook(h):
        _state["hook"] = h
        _state["set"] = True

    def get_axon_ntff_profile_hook():
        if not _state["set"]:
            set_axon_ntff_profile_hook(_make_hook())
        return _state["hook"]

    mod.set_axon_ntff_profile_hook = set_axon_ntff_profile_hook
    mod.get_axon_ntff_profile_hook = get_axon_ntff_profile_hook
    sys.modules["antenv.axon_hooks"] = mod
    antenv.axon_hooks = mod


def _build_nc():
    import concourse.bass as bass
    import concourse.tile as tile
    from concourse import bacc, mybir

    dtb = mybir.dt.bfloat16
    dtf = mybir.dt.float32
    Exp = mybir.ActivationFunctionType.Exp

    nc = bacc.Bacc(None, target_bir_lowering=False)

    xt = nc.dram_tensor("xt", [BPC, C, N], dtb, kind="ExternalInput")
    wq = nc.dram_tensor("wqkvT", [C, 3 * C], dtb, kind="ExternalInput")
    wp = nc.dram_tensor("wprojT", [C, C], dtb, kind="ExternalInput")
    bqk = nc.dram_tensor("bqk", [P, 16], dtf, kind="ExternalInput")
    bv = nc.dram_tensor("bv", [C], dtb, kind="ExternalInput")
    bpr = nc.dram_tensor("bproj", [C], dtb, kind="ExternalInput")
    y = nc.dram_tensor("y", [BPC, N, C], dtf, kind="ExternalOutput")

    from contextlib import ExitStack

    with tile.TileContext(nc) as tc:
        with ExitStack() as ctx:
            consts = ctx.enter_context(tc.tile_pool(name="consts", bufs=1))
            wpool = ctx.enter_context(tc.tile_pool(name="weights", bufs=1))
            xpool = ctx.enter_context(tc.tile_pool(name="xin", bufs=2))
            qkpool = ctx.enter_context(tc.tile_pool(name="qk", bufs=2))
            vpool = ctx.enter_context(tc.tile_pool(name="vv", bufs=2))
            epool = ctx.enter_context(tc.tile_pool(name="est", bufs=4))
            opool = ctx.enter_context(tc.tile_pool(name="ot", bufs=2))
            n2 = ctx.enter_context(tc.tile_pool(name="n2", bufs=3))
            n1 = ctx.enter_context(tc.tile_pool(name="n1", bufs=2))
            outpool = ctx.enter_context(tc.tile_pool(name="outs", bufs=2))
            psA = ctx.enter_context(tc.tile_pool(name="psA", bufs=3, space="PSUM"))
            psB = ctx.enter_context(tc.tile_pool(name="psB", bufs=2, space="PSUM"))

            wq_sb = wpool.tile([P, CT, 3 * C], dtb, tag="wq")
            wp_sb = wpool.tile([P, CT, C], dtb, tag="wp")
            bqk_sb = consts.tile([P, 16], dtf, tag="bqk")
            ones1 = consts.tile([1, P], dtb, tag="ones1")
            nc.vector.memset(ones1[:], 1.0)
            bvb_sb = consts.tile([P, C], dtb, tag="bvb")
            bpb_sb = consts.tile([P, C], dtb, tag="bpb")

            def load_x(b):
                x_sb = xpool.tile([P, CT, N], dtb, tag="x")
                xb = xt[b].rearrange("(ct p) n -> p ct n", p=P)
                for ct in range(CT):
                    eng = nc.sync if ct % 2 == 0 else nc.gpsimd
                    eng.dma_start(out=x_sb[:, ct], in_=xb[:, ct])
                return x_sb

            def emit_qk_tile(x_sb, qk_sb, ot, wide):
                """qT/kT o-tile: psum [o,n] accumulated over ct, DVE evac+bias.
                wide=True uses one 2-bank psA tile + single evac (phase 1,
                when psA is otherwise idle); wide=False uses two 1-bank psB
                tiles (attention-phase filler)."""
                if wide:
                    ps = psA.tile([P, 640], dtf, tag="psA")
                    for (c0, cw) in NCH:
                        for ct in range(CT):
                            nc.tensor.matmul(
                                ps[:, c0:c0 + cw],
                                lhsT=wq_sb[:, ct, ot * P:(ot + 1) * P],
                                rhs=x_sb[:, ct, c0:c0 + cw],
                                start=(ct == 0),
                                stop=(ct == CT - 1),
                            )
                    nc.vector.tensor_scalar_add(
                        out=qk_sb[:, ot, :],
                        in0=ps[:, :N],
                        scalar1=bqk_sb[:, ot:ot + 1],
                    )
                    return
                for (c0, cw) in NCH:
                    ps = psB.tile([P, 512], dtf, tag="psB")
                    for ct in range(CT):
                        nc.tensor.matmul(
                            ps[:, :cw],
                            lhsT=wq_sb[:, ct, ot * P:(ot + 1) * P],
                            rhs=x_sb[:, ct, c0:c0 + cw],
                            start=(ct == 0),
                            stop=(ct == CT - 1),
                        )
                    nc.vector.tensor_scalar_add(
                        out=qk_sb[:, ot, c0:c0 + cw],
                        in0=ps[:, :cw],
                        scalar1=bqk_sb[:, ot:ot + 1],
                    )

            def emit_v_chunk(x_sb, v_sb, nt, oc):
                """V 512-col chunk: psum [n,o], scatter into per-head 65-slots."""
                nh = NTS[nt]
                ps = psB.tile([P, 512], dtf, tag="psB")
                for ct in range(CT):
                    nc.tensor.matmul(
                        ps[:nh],
                        lhsT=x_sb[:, ct, nt * P:nt * P + nh],
                        rhs=wq_sb[:, ct, 2 * C + oc * 512:2 * C + (oc + 1) * 512],
                        start=(ct == 0),
                        stop=(ct == CT - 1),
                    )
                for hh in range(8):
                    h = oc * 8 + hh
                    nc.vector.tensor_add(
                        out=v_sb[:nh, nt, h * 65:h * 65 + 64],
                        in0=ps[:nh, hh * 64:(hh + 1) * 64],
                        in1=bvb_sb[:nh, h * 64:(h + 1) * 64],
                    )

            def alloc_v(b):
                v_sb = vpool.tile([P, NT, H * 65], dtb, tag="v")
                v4 = v_sb[:].rearrange("p nt (h c) -> p nt h c", c=65)
                nc.vector.memset(v4[:, :, :, 64], 1.0)
                return v_sb

            def emit_pair(qk_sb, v_sb, ot_sb, hp, act_help=True):
                """Head pair: S^T (row-group interleaved), exp, PV(+sumexp),
                fast OT' evac. Returns deferred normalize-mul thunks."""
                ob = hp
                estA = epool.tile([P, NT, N], dtb, tag="est")
                estB = epool.tile([P, NT, N], dtb, tag="est")
                for mt in range(NT):
                    mh = NTS[mt]
                    psa = psA.tile([P, 640], dtf, tag="psA")
                    psb = psA.tile([P, 640], dtf, tag="psA")
                    for (c0, cw) in NCH:
                        nc.tensor.matmul(
                            psa[:mh, c0:c0 + cw],
                            lhsT=qk_sb[0:64, 8 + ob, mt * P:mt * P + mh],
                            rhs=qk_sb[0:64, ob, c0:c0 + cw],
                        )
                    for (c0, cw) in NCH:
                        nc.tensor.matmul(
                            psb[:mh, c0:c0 + cw],
                            lhsT=qk_sb[64:128, 8 + ob, mt * P:mt * P + mh],
                            rhs=qk_sb[64:128, ob, c0:c0 + cw],
                        )
                    nc.scalar.activation(
                        out=estA[:mh, mt, :], in_=psa[:mh, :N], func=Exp, scale=0.125
                    )
                    nc.scalar.activation(
                        out=estB[:mh, mt, :], in_=psb[:mh, :N], func=Exp, scale=0.125
                    )
                muls = []
                for h, est, p0 in ((2 * hp, estA, 0), (2 * hp + 1, estB, 64)):
                    use_act_otr = act_help and p0 == 0
                    pso = psA.tile([P, 640], dtf, tag="psA")
                    for (c0, cw) in NCH:
                        for mt in range(NT):
                            mh = NTS[mt]
                            nc.tensor.matmul(
                                pso[:65, c0:c0 + cw],
                                lhsT=v_sb[:mh, mt, h * 65:h * 65 + 65],
                                rhs=est[:mh, mt, c0:c0 + cw],
                                start=(mt == 0),
                                stop=(mt == NT - 1),
                            )
                    # fast psum evac; recip must read base partition 0.
                    # s1 copy rides the ACT engine (idle during PV) so the
                    # PSUM slot frees quickly without queueing behind DVE.
                    otr = n2.tile([64, N], dtb, tag="otr")
                    if use_act_otr:
                        nc.scalar.copy(out=otr[:], in_=pso[:64, :N])
                    else:
                        nc.vector.tensor_copy(out=otr[:], in_=pso[:64, :N])
                    s1 = n1.tile([1, N], dtf, tag="s1")
                    if act_help:
                        nc.scalar.copy(out=s1[0:1, :], in_=pso[64:65, :N])
                    else:
                        nc.vector.tensor_copy(out=s1[0:1, :], in_=pso[64:65, :N])
                    rec = n1.tile([1, N], dtf, tag="rec")
                    nc.vector.reciprocal_approx_fast(out=rec[0:1, :], in_=s1[0:1, :])
                    recb = n2.tile([64, N], dtf, tag="recb")
                    nc.gpsimd.partition_broadcast(recb[:], rec[0:1, :])

                    def mk(p0=p0, ob=ob, otr=otr, recb=recb):
                        nc.vector.tensor_mul(
                            out=ot_sb[p0:p0 + 64, ob, :], in0=otr[:], in1=recb[:]
                        )

                    muls.append(mk)
                return muls

            def emit_proj_seg(ot_sb, b, nt, oc, ps, seg, evac_act=False):
                nh = NTS[nt]
                for ct in range(seg * 4, seg * 4 + 4):
                    nc.tensor.matmul(
                        ps[:nh],
                        lhsT=ot_sb[:, ct, nt * P:nt * P + nh],
                        rhs=wp_sb[:, ct, oc * 512:(oc + 1) * 512],
                        start=(ct == 0),
                        stop=(ct == CT - 1 and not evac_act),
                    )
                if seg == 1:
                    outt = outpool.tile([P, 512], dtf, tag="out")
                    if evac_act:
                        # fold bias in as a K=1 ones-row matmul, evac on the
                        # tail-idle ACT engine (DVE is busy with normalize)
                        nc.tensor.matmul(
                            ps[:nh],
                            lhsT=ones1[0:1, :nh],
                            rhs=bpb_sb[0:1, oc * 512:(oc + 1) * 512],
                            start=False,
                            stop=True,
                        )
                        nc.scalar.copy(out=outt[:nh], in_=ps[:nh])
                    else:
                        nc.vector.tensor_add(
                            out=outt[:nh],
                            in0=ps[:nh],
                            in1=bpb_sb[:nh, oc * 512:(oc + 1) * 512],
                        )
                    nc.sync.dma_start(
                        out=y[b, nt * P:nt * P + nh, oc * 512:(oc + 1) * 512],
                        in_=outt[:nh],
                    )

            def emit_proj_chunk(ot_sb, b, nt, oc, wide=False, evac_act=False):
                if wide:
                    pw = psA.tile([P, 640], dtf, tag="psA")
                    ps = pw[:, :512]
                else:
                    ps = psB.tile([P, 512], dtf, tag="psB")
                emit_proj_seg(ot_sb, b, nt, oc, ps, 0, evac_act)
                emit_proj_seg(ot_sb, b, nt, oc, ps, 1, evac_act)

            # ---- phase 0: input DMAs in first-needed order ----
            x0 = load_x(0)
            for (g0, g1) in [(0, 256), (256, 512), (512, 1024), (1024, 1536),
                             (1536, 2048)]:
                for ct in range(CT):
                    eng = nc.gpsimd if ct % 2 == 0 else nc.sync
                    eng.dma_start(
                        out=wq_sb[:, ct, g0:g1],
                        in_=wq[ct * P:(ct + 1) * P, g0:g1],
                    )
                if g0 == 0:
                    nc.sync.dma_start(out=bqk_sb[:], in_=bqk[:])
            for ct in range(CT):
                nc.sync.dma_start(
                    out=wq_sb[:, ct, 2 * C:], in_=wq[ct * P:(ct + 1) * P, 2 * C:]
                )
            nc.sync.dma_start(
                out=bvb_sb[:], in_=bass.AP(tensor=bv, offset=0, ap=[[0, P], [1, C]])
            )
            for ct in range(CT):
                nc.sync.dma_start(out=wp_sb[:, ct], in_=wp[ct * P:(ct + 1) * P, :])
            nc.sync.dma_start(
                out=bpb_sb[:], in_=bass.AP(tensor=bpr, offset=0, ap=[[0, P], [1, C]])
            )

            # ---- phase 1: QKV(b0) + V(b0), dense ----
            qk0 = qkpool.tile([P, 16, N], dtb, tag="qk")
            for ot in range(16):
                emit_qk_tile(x0, qk0, ot, wide=True)
            v0 = alloc_v(0)
            for nt in range(NT):
                for oc in range(2):
                    emit_v_chunk(x0, v0, nt, oc)

            # ---- phase 2: attention(b0) with QKV(b1)+V(b1) matmuls as PE
            # fillers between head pairs (keeps the PE dense and HAM warm) ----
            x1 = load_x(1)
            qk1 = qkpool.tile([P, 16, N], dtb, tag="qk")
            v1 = alloc_v(1)
            ot0 = opool.tile([P, CT, N], dtb, tag="ot")

            fillers = [
                lambda ot=ot: emit_qk_tile(x1, qk1, ot, wide=False)
                for ot in range(16)
            ]
            fillers += [
                lambda nt=nt: emit_v_chunk(x1, v1, nt, 0) for nt in range(NT)
            ]
            per = [3, 3, 3, 3, 3, 2, 2, 2]
            fi = 0
            for hp in range(H // 2):
                muls = emit_pair(qk0, v0, ot0, hp, act_help=(hp < 5))
                for _ in range(per[hp]):
                    fillers[fi]()
                    fi += 1
                for m in muls:
                    m()

            # ---- phase 3: attention(b1) with proj(b0) fillers ----
            ot1 = opool.tile([P, CT, N], dtb, tag="ot")
            fillers = [
                lambda nt=nt: emit_v_chunk(x1, v1, nt, 1) for nt in range(NT)
            ]
            fillers += [
                lambda nt=nt, oc=oc: emit_proj_chunk(ot0, 0, nt, oc)
                for nt in range(NT)
                for oc in range(2)
            ]
            per = [3, 3, 2, 1, 1, 1, 2, 2]
            fi = 0
            for hp in range(H // 2):
                muls = emit_pair(qk1, v1, ot1, hp, act_help=False)
                for _ in range(per[hp]):
                    fillers[fi]()
                    fi += 1
                for m in muls:
                    m()

            # ---- phase 4: proj(b1) tail ----
            for i, (nt, oc) in enumerate(
                [(nt, oc) for nt in range(NT) for oc in range(2)]
            ):
                emit_proj_chunk(ot1, 1, nt, oc, wide=(i % 2 == 0))
    nc.compile()
    return nc


def kernel(x, w_qkv, b_qkv, w_proj, b_proj):
    global LAST_RESULT
    _ensure_ntff_hook()
    from concourse.bass_utils import run_bass_kernel_spmd

    bf16 = ml_dtypes.bfloat16
    x = np.asarray(x, dtype=np.float32)
    w_qkv = np.asarray(w_qkv, dtype=np.float32)
    b_qkv = np.asarray(b_qkv, dtype=np.float32)
    w_proj = np.asarray(w_proj, dtype=np.float32)
    b_proj = np.asarray(b_proj, dtype=np.float32)

    xT = np.ascontiguousarray(np.transpose(x, (0, 2, 1))).astype(bf16)  # [B, C, N]
    wqkvT = np.ascontiguousarray(w_qkv.T).astype(bf16)  # [C, 3C]
    wprojT = np.ascontiguousarray(w_proj.T).astype(bf16)  # [C, C]
    bqk = np.ascontiguousarray(b_qkv[:2 * C].reshape(16, P).T).astype(np.float32)
    bv = np.ascontiguousarray(b_qkv[2 * C:]).astype(bf16)
    bpr = np.ascontiguousarray(b_proj).astype(bf16)

    in_maps = []
    for i in range(NCORES):
        in_maps.append(
            {
                "xt": np.ascontiguousarray(xT[i * BPC:(i + 1) * BPC]),
                "wqkvT": wqkvT,
                "wprojT": wprojT,
                "bqk": bqk,
                "bv": bv,
                "bproj": bpr,
            }
        )

    if "nc" not in _CACHE:
        _CACHE["nc"] = _build_nc()
    nc = _CACHE["nc"]

    res = run_bass_kernel_spmd(nc, in_maps, core_ids=list(range(NCORES)))
    LAST_RESULT = res
    out = np.concatenate([r["y"] for r in res.results], axis=0)
    return np.ascontiguousarray(out.astype(np.float32))


if __name__ == "__main__":
    rng = np.random.default_rng(0)
    x = rng.standard_normal((B, N, C), dtype=np.float32)
    w_qkv = rng.standard_normal((3 * C, C), dtype=np.float32) * C ** -0.5
    b_qkv = rng.standard_normal(3 * C).astype(np.float32) * 0.02
    w_proj = rng.standard_normal((C, C), dtype=np.float32) * C ** -0.5
    b_proj = rng.standard_normal(C).astype(np.float32) * 0.02
    out = kernel(x=x, w_qkv=w_qkv, b_qkv=b_qkv, w_proj=w_proj, b_proj=b_proj)
    print(out.shape, out.dtype)
